# revision 46
# baseline (speedup 1.0000x reference)
"""Trainium2 Bass kernel for nn_CBAM_84799834292534.

Strategy:
- 8 cores = 4 batch samples x 2 vertical halves. Half-1 cores receive
  row-flipped inputs/weights so every core runs the identical program
  ("local top" = its outer image edge, halo rows toward the cut edge).
- Halos handled by redundant compute (no halo exchange).
- SFOM's DCT gating collapses analytically: idct(gate*dct(x)) == gate*x,
  and mean(dct(x)) == dot(x, w) with w = idct_ortho(ones)/N.
- Convs are shifted matmuls with channels on partitions, bf16 operands
  (fp32 PSUM accumulate). K-packing (dj-shifts along the contraction dim
  via shifted input copies) and M-packing (dj-shifts along output
  channels with shifted PSUM adds) keep the PE near full utilization.
  Weight-major groups of 6 tiles share each stationary operand.
- InstanceNorm/BatchNorm/DCT-mean stats use per-tile bn_stats + tiny
  AllReduces (pair groups for per-sample stats, all-8 for BatchNorm).
  Stats fire per tile so the AllReduce launches right after the last
  band tile; halo tiles then overlap the collective. ReLU + K-pack
  copies are row-banded so next-layer matmuls start after band 0.
- L6 (64->32) output is stacked x4 across partitions: h6s[32g+c, T, :]
  holds tile t=4T+g, so relu/sigmoid/bn_stats run on 128 partitions
  (4x fewer cols each) and one block-diagonal matmul computes the 1x1
  conv for 4 tiles at once. The finale (broadcast via e_g x ones
  stationaries, multiply with f, store) is pipelined after the sigmoids.
- All bulk DMA (weights, dct-mean vector) happens in the startup window:
  mid-kernel DMA descriptor streams slow the PE's SBUF streaming and a
  clogged queue delays the IN1 collective's input DMA.
- NOTE (measured): a NEFF containing any collective whose replica group
  crosses the fixed core pairs (0,1)(2,3)(4,5)(6,7) caps PE streaming at
  ~1.93 rows/ns (~2.0GHz) for the WHOLE program; the all-8 BatchNorm
  AllReduce makes this unavoidable here. Pair-only collectives run at
  the full ~2.4GHz. Per-sample (local) BN stats would lift the cap but
  cost ~3e-2 relative error -- over the 2e-2 gate.
- NOTE (measured): core launch skew is 8-22us and variable run-to-run;
  it lands at the first all-8 collective (L1's BN AR) and at IN1's pair
  AR. A gating startup sync costs more than it saves (own latency ~15us
  + return DMA competing with the input stream).
- PSUM: all 8 banks in one rotating pool (7 conv bufs) -- the 7th buf
  removed ~2.4us PE bubbles at conv-group boundaries.
"""
import sys
import types

sys.path.insert(0, '/opt/trn_rl_repo')
import numpy as np
import ml_dtypes

BF16 = ml_dtypes.bfloat16

B, C, H0, W0 = 4, 128, 128, 128
H1 = 130          # after conv1
H2 = 132          # after conv2 (final spatial)
ST = 138          # uniform row stride of on-chip layouts
EPS = 1e-5
NCORES = 8
HB = 66           # output band rows per core

XROWS = 84        # x rows needed per core
R_F = 84          # f (SFOM output) data rows
R_H = [81, 78, 75, 72, 69, 66]   # SPEM layer output rows (local)

CNT1_LOC = 65 * 130
CNT1_TOT = 130 * 130
CNT2_LOC = 66 * 132
CNT2_TOT = 132 * 132
CNTB_LOC = 66 * 132
CNTB_TOT = 8 * 66 * 132


def _idct_ortho_np(Xin):
    """numpy copy of the reference _idct_ortho (float64)."""
    X = np.asarray(Xin, np.float64)
    N = X.shape[-1]
    scale = np.full(N, np.sqrt(N / 2.0))
    scale[0] = np.sqrt(float(N))
    Xv = X * scale
    k = np.arange(N) * (np.pi / (2.0 * N))
    Wr, Wi = np.cos(k), np.sin(k)
    Vti = np.concatenate([np.zeros(1), -Xv[::-1][:-1]])
    V = (Xv * Wr - Vti * Wi) + 1j * (Xv * Wi + Vti * Wr)
    v = np.fft.ifft(V).real
    out = np.zeros_like(v)
    out[0::2] = v[: (N + 1) // 2]
    out[1::2] = v[::-1][: N // 2]
    return out


def dct_mean_weights():
    """w such that mean(dct_ortho(x)) == dot(x, w), x of length H2*W2."""
    N = H2 * H2
    return _idct_ortho_np(np.ones(N)) / N


def _install_ntff_hook():
    if "antenv.axon_hooks" in sys.modules:
        return
    mod = types.ModuleType("antenv.axon_hooks")
    _state = {"hook": None}
    mod.set_axon_ntff_profile_hook = lambda h: _state.__setitem__("hook", h)
    mod.get_axon_ntff_profile_hook = lambda: _state["hook"]
    sys.modules["antenv.axon_hooks"] = mod
    try:
        from trn_agent_boot.trn_boot import _ntff_profile_via_ctypes
        mod.set_axon_ntff_profile_hook(
            _ntff_profile_via_ctypes('/opt/axon/libaxon_pjrt.so'))
    except Exception:
        pass


# ----------------------------------------------------------------------------
# program build
# ----------------------------------------------------------------------------

_PROGRAM_CACHE = {}


class _StopBuild(Exception):
    pass


def build_program(debug_taps=False, stage_limit=99):
    key = (bool(debug_taps), stage_limit)
    if key in _PROGRAM_CACHE:
        return _PROGRAM_CACHE[key]

    import concourse.bacc as bacc
    import concourse.tile as tile
    from concourse import mybir

    f32 = mybir.dt.float32
    f32r = mybir.dt.float32r
    bf16 = mybir.dt.bfloat16
    AF = mybir.ActivationFunctionType
    AL = mybir.AluOpType
    AX = mybir.AxisListType

    nc = bacc.Bacc("TRN2", target_bir_lowering=False)

    # ---------------- external tensors ----------------
    xs_d = nc.dram_tensor("xs", [C, XROWS, W0], bf16, kind="ExternalInput")
    w1_d = nc.dram_tensor("w1", [C, 9, C], bf16, kind="ExternalInput")
    w2_d = nc.dram_tensor("w2c", [C, 9, C], bf16, kind="ExternalInput")
    s_d = [
        nc.dram_tensor("s1", [C, 14, 128], bf16, kind="ExternalInput"),
        nc.dram_tensor("s2", [C, 7, 128], bf16, kind="ExternalInput"),
        nc.dram_tensor("s3", [C, 28, 128], bf16, kind="ExternalInput"),
        nc.dram_tensor("s4", [C, 49, 128], bf16, kind="ExternalInput"),
        nc.dram_tensor("s5", [C, 28, 128], bf16, kind="ExternalInput"),
        nc.dram_tensor("s6", [C, 7, 128], bf16, kind="ExternalInput"),
    ]
    cw_d = nc.dram_tensor("cw", [C, 4], bf16, kind="ExternalInput")
    cb_d = nc.dram_tensor("cb", [4, 1], f32, kind="ExternalInput")
    ob_d = nc.dram_tensor("obk", [4, 4, 128], bf16, kind="ExternalInput")
    sa1_d = nc.dram_tensor("sa1t", [C, 8], f32, kind="ExternalInput")
    sa2_d = nc.dram_tensor("sa2t", [8, C], f32, kind="ExternalInput")
    gb_d = nc.dram_tensor("gb", [6, 2, C], f32, kind="ExternalInput")
    wv_d = nc.dram_tensor("wv", [HB, H2], bf16, kind="ExternalInput")
    out_d = nc.dram_tensor("out", [C, 22, 414], f32, kind="ExternalOutput")
    S_W = float(dct_mean_weights().sum())

    taps = {}
    if debug_taps:
        def tap(name, shape, dt=bf16):
            taps[name] = nc.dram_tensor("tap_" + name, shape, dt,
                                        kind="ExternalOutput")
        tap("xp", [C, 87, ST])
        tap("out1", [C, 84, ST])
        tap("r1p", [C, 88, ST])
        tap("out2", [C, 84, ST])
        tap("f", [C, 89, ST])
        tap("stats1", [C, 2], f32)
        tap("stats2", [C, 3], f32)
        tap("gate", [C, 1], f32)
        for k in range(5):
            tap(f"h{k+1}", [128, R_H[k] + 5, ST])
        tap("h6", [C, 6, 414])

    PAIRS = [[0, 1], [2, 3], [4, 5], [6, 7]]
    ALL8 = [list(range(NCORES))]

    with tile.TileContext(nc) as tc:
        stage = tc.alloc_tile_pool(name="stage", bufs=3)
        fpool = tc.alloc_tile_pool(name="fpool", bufs=1)
        wts = tc.alloc_tile_pool(name="wts", bufs=1)
        cons = tc.alloc_tile_pool(name="cons", bufs=1)
        sm = tc.alloc_tile_pool(name="sm", bufs=2)
        smc = tc.alloc_tile_pool(name="smc", bufs=1)
        wvp = tc.alloc_tile_pool(name="wvp", bufs=2)
        otp = tc.alloc_tile_pool(name="otp", bufs=6)
        sap = tc.alloc_tile_pool(name="sap", bufs=3)
        cps = tc.alloc_tile_pool(name="cps", bufs=7, space="PSUM")
        drp = tc.alloc_tile_pool(name="drp", bufs=1, space="DRAM")

        def flat(t):
            return t.rearrange("p r c -> p (r c)")

        def ckpt(n):
            if stage_limit <= n:
                raise _StopBuild()

        # ---------------- constants ----------------
        sa1_sb = cons.tile([C, 8], f32, tag="sa1")
        sa2_sb = cons.tile([8, C], f32, tag="sa2")
        cw_sb = cons.tile([C, 4], bf16, tag="cw")
        cb_sb = cons.tile([4, 1], f32, tag="cb")
        gb_sb = cons.tile([C, 6, 2], f32, tag="gb")
        eps_sb = cons.tile([C, 1], f32, tag="eps")
        nc.vector.memset(eps_sb, EPS)
        nc.sync.dma_start(out=sa1_sb, in_=sa1_d[:, :])
        nc.sync.dma_start(out=sa2_sb, in_=sa2_d[:, :])
        nc.sync.dma_start(out=cw_sb, in_=cw_d[:, :])
        nc.sync.dma_start(out=cb_sb, in_=cb_d[:, :])
        nc.sync.dma_start(out=gb_sb, in_=gb_d[:, :, :].transpose([2, 0, 1]))
        # prep stationaries: ob[:, g, :] is e_g x ones(128) -> broadcast row g
        ob_sb = cons.tile([4, 4, 128], bf16, tag="ob")
        nc.sync.dma_start(out=ob_sb, in_=ob_d[:, :, :])

        def load_weights(dram, nsl, cols, tag):
            wt = wts.tile([C, nsl, cols], bf16, tag=tag, name=f"wt_{tag}")
            nc.sync.dma_start(out=wt, in_=dram[:, :, :])
            return wt

        # ---------------- generic helpers ----------------
        def all_reduce(sb_in, k, groups, p=C):
            n = len(_ar_cnt)
            _ar_cnt.append(0)
            din = drp.tile([p, k], f32, tag=f"arin{n}")
            dout = drp.tile([p, k], f32, tag=f"arout{n}")
            nc.sync.dma_start(out=din, in_=sb_in)
            nc.gpsimd.collective_compute(
                "AllReduce", AL.add, replica_groups=groups,
                ins=[din[:, :].opt()], outs=[dout[:, :].opt()])
            sb_out = smc.tile([p, k], f32, tag=f"ar{n}")
            nc.sync.dma_start(out=sb_out, in_=dout)
            return sb_out

        _ar_cnt = []

        def sums_from_mv(mv, count, p=C):
            """mv [p,2] (mean, biased var) -> packed [p,2] (sum, sum_sq)."""
            pk = smc.tile([p, 2], f32, tag=f"pk{len(_pk_cnt)}")
            _pk_cnt.append(0)
            nc.vector.tensor_scalar_mul(out=pk[:, 0:1], in0=mv[:, 0:1],
                                        scalar1=float(count))
            # e2 = (var + mean^2) * count
            nc.vector.tensor_mul(out=pk[:, 1:2], in0=mv[:, 0:1], in1=mv[:, 0:1])
            nc.vector.tensor_add(out=pk[:, 1:2], in0=pk[:, 1:2], in1=mv[:, 1:2])
            nc.vector.tensor_scalar_mul(out=pk[:, 1:2], in0=pk[:, 1:2],
                                        scalar1=float(count))
            return pk

        def mu_rstd_from_sums(gl, total, p=C):
            """gl [p,2] global (sum, sumsq) -> (mu [p,1], rstd [p,1])."""
            n = len(_mr_cnt)
            _mr_cnt.append(0)
            mu = smc.tile([p, 1], f32, tag=f"mu{n}")
            rs = smc.tile([p, 1], f32, tag=f"rs{n}")
            tv = smc.tile([p, 1], f32, tag=f"tv{n}")
            nc.vector.tensor_scalar_mul(out=mu, in0=gl[:, 0:1],
                                        scalar1=1.0 / total)
            nc.vector.tensor_scalar(out=tv, in0=mu, scalar1=mu,
                                    scalar2=-1.0, op0=AL.mult, op1=AL.mult)
            nc.vector.tensor_scalar(out=tv, in0=gl[:, 1:2],
                                    scalar1=1.0 / total, scalar2=tv,
                                    op0=AL.mult, op1=AL.add)   # var
            nc.scalar.activation(out=tv, in_=tv, func=AF.Sqrt,
                                 bias=eps_sb[0:p, :], scale=1.0)
            nc.vector.reciprocal(out=rs, in_=tv)
            return mu, rs

        _pk_cnt = []
        _mr_cnt = []

        def bn_flat_stats(src_f32, p, flat_start, flat_len):
            """bn_stats over a contiguous flat span (pads must be zeroed;
            zeros only dilute mean/var, raw sums are unaffected)."""
            sf = flat(src_f32)
            nchunks = (flat_len + 511) // 512
            stats = sm.tile([p, nchunks, 6], f32, tag="st")
            for j in range(nchunks):
                a = flat_start + 512 * j
                b = min(flat_start + flat_len, a + 512)
                nc.vector.bn_stats(out=stats[:, j, :], in_=sf[0:p, a:b])
            mv = sm.tile([p, 2], f32, tag="mv")
            nc.vector.bn_aggr(out=mv, in_=stats)
            return mv

        def _build_body():
            # ================= stage 0: input build =================
            # reflect-pad layout built by direct strided DMAs; zeros only on
            # the border cells the shifted conv reads can touch. Input DMAs
            # are split across chunks so conv1 can start on the first rows.
            xp = stage.tile([C, 87, ST], bf16, tag="stage")
            nc.vector.memset(xp[:, 0:1, :], 0.0)
            nc.vector.memset(xp[:, 86:87, :], 0.0)
            nc.vector.memset(xp[:, 1:86, 0:1], 0.0)
            nc.vector.memset(xp[:, 1:86, 131:138], 0.0)
            w1_sb = load_weights(w1_d, 9, C, "w1")
            nc.sync.dma_start(out=xp[:, 1:2, 2:130], in_=xs_d[:, 1:2, :])
            nc.sync.dma_start(out=xp[:, 1:2, 1:2], in_=xs_d[:, 1:2, 1:2])
            nc.sync.dma_start(out=xp[:, 1:2, 130:131],
                              in_=xs_d[:, 1:2, 126:127])
            for r0 in range(0, 84, 21):
                r1 = r0 + 21
                nc.sync.dma_start(out=xp[:, 2 + r0:2 + r1, 2:130],
                                  in_=xs_d[:, r0:r1, :])
                nc.sync.dma_start(out=xp[:, 2 + r0:2 + r1, 1:2],
                                  in_=xs_d[:, r0:r1, 1:2])
                nc.sync.dma_start(out=xp[:, 2 + r0:2 + r1, 130:131],
                                  in_=xs_d[:, r0:r1, 126:127])

            # ====== preload every weight + the dct-mean vector right after
            # the input: mid-kernel DMA descriptor streams throttle the PE's
            # SBUF reads, so all bulk loads happen in the startup window.
            w2_sb = load_weights(w2_d, 9, C, "w2")
            s1_sb = load_weights(s_d[0], 14, 128, "s1")
            s2_sb = load_weights(s_d[1], 7, 128, "s2")
            s3_sb = load_weights(s_d[2], 28, 128, "s3")
            s4_sb = load_weights(s_d[3], 49, 128, "s4")
            s5_sb = load_weights(s_d[4], 28, 128, "s5")
            s6_sb = load_weights(s_d[5], 7, 128, "s6")
            # dct-mean vector also loads in the startup window: putting it in
            # the IN1-AR stall clogged the DMA path and delayed the collective
            # input by ~7us.
            wvfull = fpool.tile([C, HB, H2], bf16, tag="wvfull")
            nc.sync.dma_start(out=wvfull,
                              in_=wv_d[:, :].partition_broadcast(C))
            if debug_taps:
                nc.sync.dma_start(out=taps["xp"][:, :, :], in_=xp)
            ckpt(0)

            # ================= conv1 =================
            # weight-major groups: consecutive matmuls share the stationary
            # operand.
            out1 = stage.tile([C, 84, ST], bf16, tag="stage")
            nc.vector.memset(out1[:, :, 130:138], 0.0)
            xp_f = flat(xp)
            N1 = 414
            st1 = sm.tile([C, 22, 6], f32, tag="st")

            def conv1_group(ts):
                pts = [cps.tile([C, N1], f32, tag="cps", name=f"pt{t}")
                        for t in ts]
                for i in range(9):
                    di, dj = divmod(i, 3)
                    for k, t in enumerate(ts):
                        ob = 3 * t * ST
                        o = ob + di * ST + dj
                        nc.tensor.matmul(out=pts[k], lhsT=w1_sb[:, i, :],
                                         rhs=xp_f[:, o:o + N1],
                                         start=(i == 0), stop=(i == 8))
                for k, t in enumerate(ts):
                    ob = 3 * t * ST
                    ptv = pts[k].rearrange("p (r c) -> p r c", c=ST)
                    nc.scalar.copy(out=out1[:, 3 * t:3 * t + 3, 0:130],
                                   in_=ptv[:, :, 0:130])
                    # IN1 stats cover rows 0..64 only (65 rows)
                    if t < 21:
                        nc.vector.bn_stats(out=st1[:, t, :],
                                           in_=flat(out1)[:, ob:ob + N1])
                    elif t == 21:
                        nc.vector.bn_stats(out=st1[:, 21, :],
                                           in_=flat(out1)[:, 63 * ST:65 * ST])

            for g in ([0, 1, 2, 3, 4, 5], [6, 7, 8, 9, 10, 11],
                      [12, 13, 14, 15, 16, 17], [18, 19, 20, 21]):
                conv1_group(g)
            mv1 = sm.tile([C, 2], f32, tag="mv")
            nc.vector.bn_aggr(out=mv1, in_=st1)
            pk1 = sums_from_mv(mv1, 65 * ST)
            gl1 = all_reduce(pk1, 2, PAIRS)
            conv1_group([22, 23, 24, 25, 26, 27])
            mu1, rs1 = mu_rstd_from_sums(gl1, CNT1_TOT)
            if debug_taps:
                nc.sync.dma_start(out=taps["out1"][:, :, :], in_=out1)
            ckpt(1)
            if debug_taps:
                nc.sync.dma_start(out=taps["stats1"][:, :], in_=gl1)
            ckpt(2)

            # negated bias for ACT: relu(x*rs1 - mu1*rs1)
            nb1 = smc.tile([C, 1], f32, tag="nb1")
            nc.vector.tensor_scalar(out=nb1, in0=mu1, scalar1=rs1,
                                    scalar2=-1.0, op0=AL.mult, op1=AL.mult)

            # ================= r1p build (banded) =================
            r1p = stage.tile([C, 88, ST], bf16, tag="stage")
            nc.vector.memset(r1p[:, 0:2, :], 0.0)
            nc.vector.memset(r1p[:, 2:87, 0:1], 0.0)
            nc.vector.memset(r1p[:, 2:87, 133:138], 0.0)

            def rel(dst, src):
                nc.scalar.activation(out=dst, in_=src, func=AF.Relu,
                                     bias=nb1, scale=rs1)

            rel(r1p[:, 2:3, 1:2], out1[:, 1:2, 1:2])
            rel(r1p[:, 2:3, 2:132], out1[:, 1:2, 0:130])
            rel(r1p[:, 2:3, 132:133], out1[:, 1:2, 128:129])
            rel(r1p[:, 3:10, 1:2], out1[:, 0:7, 1:2])
            rel(r1p[:, 3:10, 132:133], out1[:, 0:7, 128:129])
            rel(r1p[:, 3:10, 2:132], out1[:, 0:7, 0:130])
            rel(r1p[:, 10:24, 2:132], out1[:, 7:21, 0:130])
            rel(r1p[:, 10:30, 1:2], out1[:, 7:27, 1:2])
            rel(r1p[:, 10:30, 132:133], out1[:, 7:27, 128:129])
            for a in range(21, 84, 21):
                rel(r1p[:, 3 + a:3 + a + 21, 2:132],
                    out1[:, a:a + 21, 0:130])
            rel(r1p[:, 30:87, 1:2], out1[:, 27:84, 1:2])
            rel(r1p[:, 30:87, 132:133], out1[:, 27:84, 128:129])
            if debug_taps:
                nc.sync.dma_start(out=taps["r1p"][:, :, :], in_=r1p)
            ckpt(3)

            # ================= conv2 (+ dct-mean dot per tile) ============
            out2 = stage.tile([C, 84, ST], bf16, tag="stage")
            nc.vector.memset(out2[:, :, 0:3], 0.0)
            nc.vector.memset(out2[:, :, 135:138], 0.0)
            r1_f = flat(r1p)
            st2 = sm.tile([C, 22, 6], f32, tag="st")
            acc = sm.tile([C, 22], f32, tag="dotacc")

            def conv2_group(ts):
                pts = [cps.tile([C, N1], f32, tag="cps", name=f"pt{t}")
                        for t in ts]
                for i in range(9):
                    di, dj = divmod(i, 3)
                    off = (di + 1) * ST + (dj - 3)
                    for k, t in enumerate(ts):
                        o = 3 * t * ST + off
                        nc.tensor.matmul(out=pts[k], lhsT=w2_sb[:, i, :],
                                         rhs=r1_f[:, o:o + N1],
                                         start=(i == 0), stop=(i == 8))
                for k, t in enumerate(ts):
                    ob = 3 * t * ST
                    ptv = pts[k].rearrange("p (r c) -> p r c", c=ST)
                    nc.scalar.copy(out=out2[:, 3 * t:3 * t + 3, 3:135],
                                   in_=ptv[:, :, 3:135])
                    if t < 22:
                        nc.vector.bn_stats(out=st2[:, t, :],
                                           in_=flat(out2)[:, ob:ob + N1])
                        scr = wvp.tile([C, 3, H2], f32, tag="scr")
                        nc.vector.tensor_mul(out=scr,
                                             in0=out2[:, 3 * t:3 * t + 3, 3:135],
                                             in1=wvfull[:, 3 * t:3 * t + 3, :])
                        nc.vector.tensor_reduce(out=acc[:, t:t + 1], in_=scr,
                                                axis=AX.XY, op=AL.add)

            for g in ([0, 1], [2, 3, 4, 5, 6, 7], [8, 9, 10, 11, 12, 13],
                      [14, 15, 16, 17, 18, 19], [20, 21]):
                conv2_group(g)
            mv2 = sm.tile([C, 2], f32, tag="mv")
            nc.vector.bn_aggr(out=mv2, in_=st2)
            dotw = smc.tile([C, 1], f32, tag="dotw")
            nc.vector.tensor_reduce(out=dotw, in_=acc, axis=AX.X, op=AL.add)
            pk2 = sums_from_mv(mv2, 66 * ST)
            pk2b = smc.tile([C, 3], f32, tag="pk2b")
            nc.vector.tensor_copy(out=pk2b[:, 0:2], in_=pk2)
            nc.vector.tensor_copy(out=pk2b[:, 2:3], in_=dotw)
            gl2 = all_reduce(pk2b, 3, PAIRS)
            conv2_group([22, 23, 24, 25, 26, 27])
            mu2, rs2 = mu_rstd_from_sums(gl2, CNT2_TOT)
            if debug_taps:
                nc.sync.dma_start(out=taps["out2"][:, :, :], in_=out2)
            ckpt(4)
            if debug_taps:
                nc.sync.dma_start(out=taps["stats2"][:, :], in_=gl2)

            # ================= SFOM gate =================
            # m = rs2 * (dotw_glob - mu2 * S_w)
            m_sb = smc.tile([C, 1], f32, tag="m")
            nc.vector.tensor_scalar(out=m_sb, in0=mu2, scalar1=-S_W,
                                    scalar2=gl2[:, 2:3], op0=AL.mult,
                                    op1=AL.add)
            nc.vector.tensor_mul(out=m_sb, in0=m_sb, in1=rs2)
            # gate = sigmoid(relu(m @ sa1) @ sa2)
            p_r = cps.tile([8, 1], f32, tag="cps", name="p_r")
            nc.tensor.matmul(out=p_r, lhsT=sa1_sb, rhs=m_sb, start=True, stop=True)
            relu_sb = smc.tile([8, 1], f32, tag="relu8")
            nc.scalar.activation(out=relu_sb, in_=p_r, func=AF.Relu,
                                 bias=0.0, scale=1.0)
            p_g = cps.tile([C, 1], f32, tag="cps", name="p_g")
            nc.tensor.matmul(out=p_g, lhsT=sa2_sb, rhs=relu_sb,
                             start=True, stop=True)
            gate = smc.tile([C, 1], f32, tag="gate")
            nc.scalar.activation(out=gate, in_=p_g, func=AF.Sigmoid,
                                 bias=0.0, scale=1.0)
            if debug_taps:
                nc.sync.dma_start(out=taps["gate"][:, :], in_=gate)
            ckpt(5)
            # s_sig = rs2 * (1+gate)/2
            ssig = smc.tile([C, 1], f32, tag="ssig")
            nc.vector.tensor_scalar(out=ssig, in0=gate, scalar1=0.5, scalar2=0.5,
                                    op0=AL.mult, op1=AL.add)
            nc.vector.tensor_mul(out=ssig, in0=ssig, in1=rs2)
            nbs = smc.tile([C, 1], f32, tag="nbs")     # -mu2*ssig
            nc.vector.tensor_scalar(out=nbs, in0=mu2, scalar1=ssig,
                                    scalar2=-1.0, op0=AL.mult, op1=AL.mult)

            # ================= SFOM apply (banded) =================
            # o2 = (out2-mu2)*rs2 ; f = sigmoid(o2*g2')*o2  (slots +4 rows)
            o2 = stage.tile([C, 84, ST], bf16, tag="stage")
            ftile = fpool.tile([C, R_F + 5, ST], bf16, tag="f")
            fr = ftile
            nc.vector.memset(ftile[:, 0:4, :], 0.0)
            nc.vector.memset(ftile[:, 88:89, :], 0.0)
            nc.vector.memset(ftile[:, 4:88, 0:3], 0.0)
            nc.vector.memset(ftile[:, 4:88, 135:138], 0.0)
            fbands = [0, 10, 21, 42, 63, 84]
            for a, e in zip(fbands, fbands[1:]):
                nc.vector.tensor_scalar(out=o2[:, a:e, 3:135],
                                        in0=out2[:, a:e, 3:135],
                                        scalar1=mu2, scalar2=rs2,
                                        op0=AL.subtract, op1=AL.mult)
                nc.scalar.activation(out=fr[:, 4 + a:4 + e, 3:135],
                                     in_=out2[:, a:e, 3:135],
                                     func=AF.Sigmoid, bias=nbs, scale=ssig)
                nc.vector.tensor_mul(out=fr[:, 4 + a:4 + e, 3:135],
                                     in0=ftile[:, 4 + a:4 + e, 3:135],
                                     in1=o2[:, a:e, 3:135])
            if debug_taps:
                nc.sync.dma_start(out=taps["f"][:, :, :], in_=ftile)
            ckpt(6)

            # ================= SPEM layers =================
            def spem_layer(lidx, src_r, wtile, co, R, NP, mms, combine, ncopies,
                           copy_cp, post_band=None):
                """One SPEM conv layer: weight-major matmul groups, strided
                psum drains, per-tile BN stats, early AR, banded relu +
                K-pack copies."""
                S = R + 5
                P = 128 if (ncopies or co > 64) else co
                h = stage.tile([P, S, ST], bf16, tag="stage")
                nc.vector.memset(h[:, 0:4, :], 0.0)
                nc.vector.memset(h[:, 4 + R:S, :], 0.0)
                nc.vector.memset(h[:, 4:4 + R, 0:3], 0.0)
                nc.vector.memset(h[:, 4:4 + R, 135:138], 0.0)
                src_f = flat(src_r)
                ntiles = R // 3
                st = sm.tile([co, 22, 6], f32, tag="st")

                def conv_group(ts):
                    pts = [cps.tile([128, NP], f32, tag="cps", name=f"pt{t}")
                            for t in ts]
                    for i, (sl, beta) in enumerate(mms):
                        for k, t in enumerate(ts):
                            o = (4 + 3 * t) * ST + beta
                            nc.tensor.matmul(out=pts[k], lhsT=wtile[:, sl, :],
                                             rhs=src_f[:, o:o + NP],
                                             start=(i == 0),
                                             stop=(i == len(mms) - 1))
                    for k, t in enumerate(ts):
                        combine(pts[k], h, t)
                        if t < 22:
                            ob = (4 + 3 * t) * ST
                            nc.vector.bn_stats(
                                out=st[:, t, :],
                                in_=flat(h)[0:co, ob:ob + N1])

                for g in ([0, 1, 2, 3, 4, 5], [6, 7, 8, 9, 10, 11],
                          [12, 13, 14, 15, 16, 17], [18, 19, 20, 21]):
                    conv_group(g)
                mvb = sm.tile([co, 2], f32, tag="mv")
                nc.vector.bn_aggr(out=mvb, in_=st)
                pkb = sums_from_mv(mvb, 66 * ST, p=co)
                glb = all_reduce(pkb, 2, ALL8, p=co)
                if ntiles > 22:
                    conv_group(list(range(22, ntiles)))
                mub, rsb = mu_rstd_from_sums(glb, CNTB_TOT, p=co)
                # scale = gamma*rstd ; bias = beta - mu*scale
                sc = smc.tile([co, 1], f32, tag=f"sc{lidx}")
                bi = smc.tile([co, 1], f32, tag=f"bi{lidx}")
                nc.vector.tensor_mul(out=sc, in0=gb_sb[0:co, lidx, 0:1], in1=rsb)
                nc.vector.tensor_mul(out=bi, in0=mub, in1=sc)
                nc.vector.tensor_sub(out=bi, in0=gb_sb[0:co, lidx, 1:2], in1=bi)
                # banded relu + K-pack shifted copies (+ optional hook);
                # short first band so the next layer's matmuls start sooner
                bands = [4, 10] + list(range(25, 4 + R, 15)) + [4 + R]
                for a, e in zip(bands, bands[1:]):
                    nc.scalar.activation(out=h[0:co, a:e, 3:135],
                                         in_=h[0:co, a:e, 3:135],
                                         func=AF.Relu, bias=bi, scale=sc)
                    for g2 in range(1, ncopies + 1):
                        nc.vector.tensor_copy(
                            out=h[g2 * copy_cp:(g2 + 1) * copy_cp, a:e,
                                  0:ST - g2],
                            in_=h[0:copy_cp, a:e, g2:ST])
                    if post_band is not None:
                        post_band(h, e)
                if debug_taps:
                    tp = taps[f"h{lidx+1}"]
                    nc.sync.dma_start(out=tp[:, :, :], in_=h[0:tp.shape[0], :, :])
                return h

            def drain_act(pt, h, t):
                ptv = pt.rearrange("p (r c) -> p r c", c=ST)
                nc.scalar.copy(out=h[:, 4 + 3 * t:7 + 3 * t, 3:135],
                               in_=ptv[:, :, 3:135])

            def mk_combine(groups, cp):
                """groups: list of (psum partition group idx, col shift).
                DVE reads at most one PSUM operand: copy then accumulate."""
                def comb(pt, h, t):
                    r0 = 4 + 3 * t
                    g0, s0 = groups[0]
                    v0 = pt[g0 * cp:(g0 + 1) * cp, s0:s0 + N1].rearrange(
                        "p (r c) -> p r c", c=ST)
                    nc.scalar.copy(out=h[0:cp, r0:r0 + 3, 3:135],
                                   in_=v0[:, :, 3:135])
                    for g, s in groups[1:]:
                        v = pt[g * cp:(g + 1) * cp, s:s + N1].rearrange(
                            "p (r c) -> p r c", c=ST)
                        nc.vector.tensor_add(out=h[0:cp, r0:r0 + 3, 3:135],
                                             in0=h[0:cp, r0:r0 + 3, 3:135],
                                             in1=v[:, :, 3:135])
                return comb

            # L1: 128->32, Mpack4: psum[g*32+co] <-> out[n-g]
            mms1 = [(di * 2 + s, (di - 3) * ST + 4 * s - 3)
                    for di in range(7) for s in range(2)]
            h1 = spem_layer(0, fr, s1_sb, 32, R_H[0], 418, mms1,
                            mk_combine([(0, 0), (1, 1), (2, 2), (3, 3)], 32),
                            3, 32)
            # L2: 32->64, Kpack4 + Mpack2(supergroups +4): psum[G*64+co]<->out[n-4G]
            ckpt(7)
            mms2 = [(di, (di - 3) * ST - 3) for di in range(7)]
            h2 = spem_layer(1, h1, s2_sb, 64, R_H[1], 418, mms2,
                            mk_combine([(0, 0), (1, 4)], 64), 1, 64)
            # L3: 64->128, Kpack2: 4 dj-groups
            ckpt(8)
            mms3 = [(di * 4 + g, (di - 3) * ST + 2 * g - 3)
                    for di in range(7) for g in range(4)]
            h3 = spem_layer(2, h2, s3_sb, 128, R_H[2], 414, mms3, drain_act,
                            0, 0)
            # L4: 128->128 plain
            ckpt(9)
            mms4 = [(di * 7 + dj, (di - 3) * ST + dj - 3)
                    for di in range(7) for dj in range(7)]
            h4 = spem_layer(3, h3, s4_sb, 128, R_H[3], 414, mms4, drain_act,
                            0, 0)
            # L5: 128->64, Mpack2: psum[g*64+co] <-> out[n-g]
            ckpt(10)
            mms5 = [(di * 4 + st_, (di - 3) * ST + 2 * st_ - 3)
                    for di in range(7) for st_ in range(4)]
            h5 = spem_layer(4, h4, s5_sb, 64, R_H[4], 416, mms5,
                            mk_combine([(0, 0), (1, 1)], 64), 1, 64)
            # ===== L6: 64->32, Kpack2 + Mpack4, output STACKED x4 =====
            # h6s[32*g + c, T, :] holds tile t = 4*T + g (3 rows x 138 flat).
            # Stacking 4 tiles across the partition axis makes relu/sigmoid/
            # stats 4x cheaper and lets one matmul compute 4 tiles' 1x1 conv.
            ckpt(11)
            mms6 = [(di, (di - 3) * ST - 3) for di in range(7)]
            SL6 = 6
            h6s = stage.tile([C, SL6, 414], bf16, tag="stage")
            h6v = h6s.rearrange("p s (r c) -> p s r c", c=ST)
            nc.vector.memset(h6v[:, :, :, 0:3], 0.0)
            nc.vector.memset(h6v[:, :, :, 135:138], 0.0)
            nc.vector.memset(h6s[64:128, 5, :], 0.0)
            src6_f = flat(h5)
            st6 = sm.tile([C, SL6, 6], f32, tag="st")

            def l6_group(ts, T0, T1):
                pts = [cps.tile([128, 420], f32, tag="cps", name=f"pt{t}")
                        for t in ts]
                for i, (sl, beta) in enumerate(mms6):
                    for k, t in enumerate(ts):
                        o = (4 + 3 * t) * ST + beta
                        nc.tensor.matmul(out=pts[k], lhsT=s6_sb[:, sl, :],
                                         rhs=src6_f[:, o:o + 420],
                                         start=(i == 0), stop=(i == 6))
                for k, t in enumerate(ts):
                    g, T = t % 4, t // 4
                    dst = h6v[32 * g:32 * g + 32, T, :, 3:135]
                    v0 = pts[k][0:32, 0:414].rearrange(
                        "p (r c) -> p r c", c=ST)
                    nc.scalar.copy(out=dst, in_=v0[:, :, 3:135])
                    for gg, s in [(1, 2), (2, 4), (3, 6)]:
                        v = pts[k][32 * gg:32 * gg + 32, s:s + 414].rearrange(
                            "p (r c) -> p r c", c=ST)
                        nc.vector.tensor_add(out=dst, in0=dst,
                                             in1=v[:, :, 3:135])
                for T in range(T0, T1):
                    nc.vector.bn_stats(out=st6[:, T, :], in_=h6s[:, T, :])

            l6_group([0, 1, 2, 3, 4, 5], 0, 1)
            l6_group([6, 7, 8, 9, 10, 11], 1, 3)
            l6_group([12, 13, 14, 15, 16, 17], 3, 4)
            l6_group([18, 19, 20, 21], 4, 6)
            mv6 = sm.tile([C, 2], f32, tag="mv")
            nc.vector.bn_aggr(out=mv6, in_=st6)
            pk6 = sums_from_mv(mv6, SL6 * 414, p=C)
            pkf = smc.tile([32, 3, 2], f32, tag="pkf")
            for g in range(1, 4):
                nc.vector.tensor_copy(out=pkf[:, g - 1, :],
                                      in_=pk6[32 * g:32 * g + 32, :])
            pk32 = smc.tile([32, 2], f32, tag="pk32")
            nc.vector.tensor_add(out=pk32, in0=pk6[0:32, :], in1=pkf[:, 0, :])
            nc.vector.tensor_add(out=pk32, in0=pk32, in1=pkf[:, 1, :])
            nc.vector.tensor_add(out=pk32, in0=pk32, in1=pkf[:, 2, :])
            gl6 = all_reduce(pk32, 2, ALL8, p=32)
            mu6, rs6 = mu_rstd_from_sums(gl6, CNTB_TOT, p=32)
            sc6 = smc.tile([32, 1], f32, tag="sc6")
            bi6 = smc.tile([32, 1], f32, tag="bi6")
            nc.vector.tensor_mul(out=sc6, in0=gb_sb[0:32, 5, 0:1], in1=rs6)
            nc.vector.tensor_mul(out=bi6, in0=mu6, in1=sc6)
            nc.vector.tensor_sub(out=bi6, in0=gb_sb[0:32, 5, 1:2], in1=bi6)
            sc6s = smc.tile([C, 1], f32, tag="sc6s")
            bi6s = smc.tile([C, 1], f32, tag="bi6s")
            for g in range(4):
                nc.vector.tensor_copy(out=sc6s[32 * g:32 * g + 32], in_=sc6)
                nc.vector.tensor_copy(out=bi6s[32 * g:32 * g + 32], in_=bi6)

            # relu + 1x1 conv (4 tiles per matmul) + sigmoid; separate loops
            # keep each engine streaming instead of ACT<->PE ping-pong
            sa_s = smc.tile([4, SL6, 414], bf16, tag="sa_s")
            p7s = []
            for T in range(SL6):
                nc.scalar.activation(out=h6v[:, T, :, 3:135],
                                     in_=h6v[:, T, :, 3:135],
                                     func=AF.Relu, bias=bi6s, scale=sc6s)
            for T in range(SL6):
                p7 = cps.tile([4, 414], f32, tag="cps", name=f"p7_{T}")
                nc.tensor.matmul(out=p7, lhsT=cw_sb, rhs=h6s[:, T, :],
                                 start=True, stop=True)
                p7s.append(p7)
            for T in range(SL6):
                nc.scalar.activation(out=sa_s[:, T, :], in_=p7s[T],
                                     func=AF.Sigmoid, bias=cb_sb, scale=1.0)

            if debug_taps:
                nc.sync.dma_start(out=taps["h6"][:, :, :], in_=h6s)

            # broadcast + multiply + store; stationary grouped by g
            f_f = flat(ftile)
            for g in range(4):
                for T in range(6 if g < 2 else 5):
                    t = 4 * T + g
                    obs = (4 + 3 * t) * ST
                    prep = cps.tile([128, 414], f32, tag="cps",
                                    name=f"prep_{t}")
                    nc.tensor.matmul(out=prep, lhsT=ob_sb[:, g, :],
                                     rhs=sa_s[:, T, :], start=True, stop=True)
                    ot = otp.tile([C, 414], f32, tag="ot")
                    nc.vector.tensor_mul(out=ot, in0=prep,
                                         in1=f_f[:, obs:obs + 414])
                    nc.sync.dma_start(out=out_d[:, t, :], in_=ot)


        try:
            _build_body()
        except _StopBuild:
            pass
        for p in [drp, cps, sap, otp, wvp, smc, sm, cons, wts, fpool,
                  stage]:
            p.release()

    nc.compile()
    _PROGRAM_CACHE[key] = (nc, taps)
    return nc, taps


# ----------------------------------------------------------------------------
# host-side packing
# ----------------------------------------------------------------------------

def _pack_core_inputs(inputs, core):
    b, half = core // 2, core % 2
    flip = (half == 1)

    def fd(w):          # flip di (axis 2) of [co, ci, kh, kw]
        return w[:, :, ::-1, :] if flip else w

    x = inputs['x'][b]
    if flip:
        x = x[:, ::-1, :]
    xs = np.ascontiguousarray(x[:, 0:XROWS, :]).astype(BF16)

    w1 = fd(inputs['conv1_w'])
    w2 = fd(inputs['conv2_w'])
    w1p = np.ascontiguousarray(
        np.transpose(w1, (2, 3, 1, 0)).reshape(9, C, C).transpose(1, 0, 2),
        np.float32)
    w2p = np.ascontiguousarray(
        np.transpose(w2, (2, 3, 1, 0)).reshape(9, C, C).transpose(1, 0, 2),
        np.float32)

    dws = [fd(inputs[f'dw{i}']) for i in range(1, 7)]

    def lhsT(w, di, dj):
        return w[:, :, di, dj].T        # [ci, co]

    # s1 [C, 14, 128]: idx di*2+s ; cols g*32+co = dj=4s+g
    s1 = np.zeros((C, 14, 128), np.float32)
    for di in range(7):
        for s in range(2):
            for g in range(4):
                dj = 4 * s + g
                if dj < 7:
                    s1[:, di * 2 + s, 32 * g:32 * g + 32] = lhsT(dws[0], di, dj)
    # s2 [C, 7, 128]: idx di ; rows s*32+ci ; cols G*64+co = dj=4G+s
    s2 = np.zeros((C, 7, 128), np.float32)
    for di in range(7):
        for s in range(4):
            for G in range(2):
                dj = 4 * G + s
                if dj < 7:
                    s2[32 * s:32 * s + 32, di, 64 * G:64 * G + 64] = \
                        lhsT(dws[1], di, dj)
    # s3 [C, 28, 128]: idx di*4+grp ; rows s*64+ci ; dj=2grp+s
    s3 = np.zeros((C, 28, 128), np.float32)
    for di in range(7):
        for grp in range(4):
            for s in range(2):
                dj = 2 * grp + s
                if dj < 7:
                    s3[64 * s:64 * s + 64, di * 4 + grp, :] = \
                        lhsT(dws[2], di, dj)
    # s4 [C, 49, 128]
    s4 = np.zeros((C, 49, 128), np.float32)
    for di in range(7):
        for dj in range(7):
            s4[:, di * 7 + dj, :] = lhsT(dws[3], di, dj)
    # s5 [C, 28, 128]: idx di*4+st ; cols g*64+co = dj=2st+g
    s5 = np.zeros((C, 28, 128), np.float32)
    for di in range(7):
        for st in range(4):
            for g in range(2):
                dj = 2 * st + g
                if dj < 7:
                    s5[:, di * 4 + st, 64 * g:64 * g + 64] = \
                        lhsT(dws[4], di, dj)
    # s6 [C, 7, 128]: idx di ; rows s*64+ci ; cols G*32+co = dj=2G+s
    s6 = np.zeros((C, 7, 128), np.float32)
    for di in range(7):
        for G in range(4):
            for s in range(2):
                dj = 2 * G + s
                if dj < 7:
                    s6[64 * s:64 * s + 64, di,
                       32 * G:32 * G + 32] = lhsT(dws[5], di, dj)

    # cw block-diagonal for the stacked 1x1 conv: cwb[32g+c, g] = cw[c]
    cwv = np.asarray(inputs['spem_cw'][0, :, 0, 0], np.float32)
    cwb = np.zeros((C, 4), np.float32)
    obk = np.zeros((4, 4, 128), np.float32)
    for g in range(4):
        cwb[32 * g:32 * g + 32, g] = cwv
        obk[g, g, :] = 1.0

    wvec = dct_mean_weights().reshape(H2, H2)
    if flip:
        wv = np.ascontiguousarray(wvec[::-1, :][0:HB]).astype(BF16)
    else:
        wv = np.ascontiguousarray(wvec[0:HB]).astype(BF16)

    gb = np.zeros((6, 2, C), np.float32)
    for k in range(6):
        g = inputs[f'bg{k+1}']
        bb = inputs[f'bb{k+1}']
        gb[k, 0, :len(g)] = g
        gb[k, 1, :len(bb)] = bb

    return {
        'xs': xs, 'w1': w1p.astype(BF16), 'w2c': w2p.astype(BF16),
        's1': s1.astype(BF16), 's2': s2.astype(BF16), 's3': s3.astype(BF16),
        's4': s4.astype(BF16), 's5': s5.astype(BF16), 's6': s6.astype(BF16),
        'cw': cwb.astype(BF16),
        'cb': np.full((4, 1), float(np.asarray(inputs['spem_cb']).reshape(())),
                      np.float32),
        'obk': obk.astype(BF16),
        'sa1t': np.ascontiguousarray(inputs['sa_w1'].T, np.float32),
        'sa2t': np.ascontiguousarray(inputs['sa_w2'].T, np.float32),
        'gb': gb, 'wv': wv,
    }


def run_cores(inputs, trace=False, debug_taps=False, stage_limit=99):
    _install_ntff_hook()
    from concourse.bass_utils import run_bass_kernel_spmd
    nc, taps = build_program(debug_taps=debug_taps, stage_limit=stage_limit)
    in_maps = [_pack_core_inputs(inputs, c) for c in range(NCORES)]
    res = run_bass_kernel_spmd(nc, in_maps, list(range(NCORES)), trace=trace)
    return res


def unpack_out(o):
    """[C, 22, 414] flat tile rows -> [C, HB, H2]."""
    return np.asarray(o, np.float32).reshape(C, HB, ST)[:, :, 3:135]


def kernel(**inputs):
    res = run_cores(inputs)
    full = np.empty((B, C, H2, H2), np.float32)
    for b in range(B):
        full[b, :, 0:HB, :] = unpack_out(res.results[2 * b]["out"])
        full[b, :, HB:H2, :] = unpack_out(res.results[2 * b + 1]["out"])[:, ::-1, :]
    return full



# revision 49
# speedup vs baseline: 1.0164x; 1.0164x over previous
"""Trainium2 Bass kernel for nn_CBAM_84799834292534.

Strategy:
- 8 cores = 4 batch samples x 2 vertical halves. Half-1 cores receive
  row-flipped inputs/weights so every core runs the identical program
  ("local top" = its outer image edge, halo rows toward the cut edge).
- Halos handled by redundant compute (no halo exchange).
- SFOM's DCT gating collapses analytically: idct(gate*dct(x)) == gate*x,
  and mean(dct(x)) == dot(x, w) with w = idct_ortho(ones)/N.
- Convs are shifted matmuls with channels on partitions, bf16 operands
  (fp32 PSUM accumulate). K-packing (dj-shifts along the contraction dim
  via shifted input copies) and M-packing (dj-shifts along output
  channels with shifted PSUM adds) keep the PE near full utilization.
  Weight-major groups of 6 tiles share each stationary operand.
- InstanceNorm/BatchNorm/DCT-mean stats use per-tile bn_stats + tiny
  AllReduces (pair groups for per-sample stats, all-8 for BatchNorm).
  Stats fire per tile so the AllReduce launches right after the last
  band tile; halo tiles then overlap the collective. ReLU + K-pack
  copies are row-banded so next-layer matmuls start after band 0.
- L6 (64->32) output is stacked x4 across partitions: h6s[32g+c, T, :]
  holds tile t=4T+g, so relu/sigmoid/bn_stats run on 128 partitions
  (4x fewer cols each) and one block-diagonal matmul computes the 1x1
  conv for 4 tiles at once. The finale (broadcast via e_g x ones
  stationaries, multiply with f, store) is pipelined after the sigmoids.
- All bulk DMA (weights, dct-mean vector) happens in the startup window:
  mid-kernel DMA descriptor streams slow the PE's SBUF streaming and a
  clogged queue delays the IN1 collective's input DMA.
- NOTE (measured): a NEFF containing any collective whose replica group
  crosses the fixed core pairs (0,1)(2,3)(4,5)(6,7) caps PE streaming at
  ~1.93 rows/ns (~2.0GHz) for the WHOLE program; the all-8 BatchNorm
  AllReduce makes this unavoidable here. Pair-only collectives run at
  the full ~2.4GHz. Per-sample (local) BN stats would lift the cap but
  cost ~3e-2 relative error -- over the 2e-2 gate.
- NOTE (measured): core launch skew is 8-22us and variable run-to-run;
  it lands at the first all-8 collective (L1's BN AR) and at IN1's pair
  AR. A gating startup sync costs more than it saves (own latency ~15us
  + return DMA competing with the input stream).
- PSUM: all 8 banks in one rotating pool (7 conv bufs) -- the 7th buf
  removed ~2.4us PE bubbles at conv-group boundaries.
"""
import sys
import types

sys.path.insert(0, '/opt/trn_rl_repo')
import numpy as np
import ml_dtypes

BF16 = ml_dtypes.bfloat16

B, C, H0, W0 = 4, 128, 128, 128
H1 = 130          # after conv1
H2 = 132          # after conv2 (final spatial)
ST = 138          # uniform row stride of on-chip layouts
EPS = 1e-5
NCORES = 8
HB = 66           # output band rows per core

XROWS = 84        # x rows needed per core
R_F = 84          # f (SFOM output) data rows
R_H = [81, 78, 75, 72, 69, 66]   # SPEM layer output rows (local)

CNT1_LOC = 65 * 130
CNT1_TOT = 130 * 130
CNT2_LOC = 66 * 132
CNT2_TOT = 132 * 132
CNTB_LOC = 66 * 132
CNTB_TOT = 8 * 66 * 132


def _idct_ortho_np(Xin):
    """numpy copy of the reference _idct_ortho (float64)."""
    X = np.asarray(Xin, np.float64)
    N = X.shape[-1]
    scale = np.full(N, np.sqrt(N / 2.0))
    scale[0] = np.sqrt(float(N))
    Xv = X * scale
    k = np.arange(N) * (np.pi / (2.0 * N))
    Wr, Wi = np.cos(k), np.sin(k)
    Vti = np.concatenate([np.zeros(1), -Xv[::-1][:-1]])
    V = (Xv * Wr - Vti * Wi) + 1j * (Xv * Wi + Vti * Wr)
    v = np.fft.ifft(V).real
    out = np.zeros_like(v)
    out[0::2] = v[: (N + 1) // 2]
    out[1::2] = v[::-1][: N // 2]
    return out


def dct_mean_weights():
    """w such that mean(dct_ortho(x)) == dot(x, w), x of length H2*W2."""
    N = H2 * H2
    return _idct_ortho_np(np.ones(N)) / N


def _install_ntff_hook():
    if "antenv.axon_hooks" in sys.modules:
        return
    mod = types.ModuleType("antenv.axon_hooks")
    _state = {"hook": None}
    mod.set_axon_ntff_profile_hook = lambda h: _state.__setitem__("hook", h)
    mod.get_axon_ntff_profile_hook = lambda: _state["hook"]
    sys.modules["antenv.axon_hooks"] = mod
    try:
        from trn_agent_boot.trn_boot import _ntff_profile_via_ctypes
        mod.set_axon_ntff_profile_hook(
            _ntff_profile_via_ctypes('/opt/axon/libaxon_pjrt.so'))
    except Exception:
        pass


# ----------------------------------------------------------------------------
# program build
# ----------------------------------------------------------------------------

_PROGRAM_CACHE = {}


class _StopBuild(Exception):
    pass


def build_program(debug_taps=False, stage_limit=99):
    key = (bool(debug_taps), stage_limit)
    if key in _PROGRAM_CACHE:
        return _PROGRAM_CACHE[key]

    import concourse.bacc as bacc
    import concourse.tile as tile
    from concourse import mybir

    f32 = mybir.dt.float32
    f32r = mybir.dt.float32r
    bf16 = mybir.dt.bfloat16
    AF = mybir.ActivationFunctionType
    AL = mybir.AluOpType
    AX = mybir.AxisListType

    nc = bacc.Bacc("TRN2", target_bir_lowering=False)

    # ---------------- external tensors ----------------
    xs_d = nc.dram_tensor("xs", [C, XROWS, W0], bf16, kind="ExternalInput")
    w1_d = nc.dram_tensor("w1", [C, 9, C], bf16, kind="ExternalInput")
    w2_d = nc.dram_tensor("w2c", [C, 9, C], bf16, kind="ExternalInput")
    s_d = [
        nc.dram_tensor("s1", [C, 14, 128], bf16, kind="ExternalInput"),
        nc.dram_tensor("s2", [C, 7, 128], bf16, kind="ExternalInput"),
        nc.dram_tensor("s3", [C, 28, 128], bf16, kind="ExternalInput"),
        nc.dram_tensor("s4", [C, 49, 128], bf16, kind="ExternalInput"),
        nc.dram_tensor("s5", [C, 28, 128], bf16, kind="ExternalInput"),
        nc.dram_tensor("s6", [C, 7, 128], bf16, kind="ExternalInput"),
    ]
    cw_d = nc.dram_tensor("cw", [C, 4], bf16, kind="ExternalInput")
    cb_d = nc.dram_tensor("cb", [4, 1], f32, kind="ExternalInput")
    ob_d = nc.dram_tensor("obk", [4, 4, 128], bf16, kind="ExternalInput")
    sa1_d = nc.dram_tensor("sa1t", [C, 8], f32, kind="ExternalInput")
    sa2_d = nc.dram_tensor("sa2t", [8, C], f32, kind="ExternalInput")
    gb_d = nc.dram_tensor("gb", [6, 2, C], f32, kind="ExternalInput")
    wv_d = nc.dram_tensor("wv", [HB, H2], bf16, kind="ExternalInput")
    out_d = nc.dram_tensor("out", [C, 22, 414], f32, kind="ExternalOutput")
    S_W = float(dct_mean_weights().sum())

    taps = {}
    if debug_taps:
        def tap(name, shape, dt=bf16):
            taps[name] = nc.dram_tensor("tap_" + name, shape, dt,
                                        kind="ExternalOutput")
        tap("xp", [C, 87, ST])
        tap("out1", [C, 84, ST])
        tap("r1p", [C, 88, ST])
        tap("out2", [C, 84, ST])
        tap("f", [C, 89, ST])
        tap("stats1", [C, 2], f32)
        tap("stats2", [C, 3], f32)
        tap("gate", [C, 1], f32)
        for k in range(5):
            tap(f"h{k+1}", [128, R_H[k] + 5, ST])
        tap("h6", [C, 6, 414])

    PAIRS = [[0, 1], [2, 3], [4, 5], [6, 7]]
    ALL8 = [list(range(NCORES))]

    with tile.TileContext(nc) as tc:
        stage = tc.alloc_tile_pool(name="stage", bufs=3)
        fpool = tc.alloc_tile_pool(name="fpool", bufs=1)
        wts = tc.alloc_tile_pool(name="wts", bufs=1)
        cons = tc.alloc_tile_pool(name="cons", bufs=1)
        sm = tc.alloc_tile_pool(name="sm", bufs=2)
        smc = tc.alloc_tile_pool(name="smc", bufs=1)
        wvp = tc.alloc_tile_pool(name="wvp", bufs=2)
        otp = tc.alloc_tile_pool(name="otp", bufs=6)
        sap = tc.alloc_tile_pool(name="sap", bufs=3)
        cps = tc.alloc_tile_pool(name="cps", bufs=7, space="PSUM")
        drp = tc.alloc_tile_pool(name="drp", bufs=1, space="DRAM")

        def flat(t):
            return t.rearrange("p r c -> p (r c)")

        def ckpt(n):
            if stage_limit <= n:
                raise _StopBuild()

        # ---------------- constants ----------------
        sa1_sb = cons.tile([C, 8], f32, tag="sa1")
        sa2_sb = cons.tile([8, C], f32, tag="sa2")
        cw_sb = cons.tile([C, 4], bf16, tag="cw")
        cb_sb = cons.tile([4, 1], f32, tag="cb")
        gb_sb = cons.tile([C, 6, 2], f32, tag="gb")
        eps_sb = cons.tile([C, 1], f32, tag="eps")
        nc.vector.memset(eps_sb, EPS)
        nc.sync.dma_start(out=sa1_sb, in_=sa1_d[:, :])
        nc.sync.dma_start(out=sa2_sb, in_=sa2_d[:, :])
        nc.sync.dma_start(out=cw_sb, in_=cw_d[:, :])
        nc.sync.dma_start(out=cb_sb, in_=cb_d[:, :])
        nc.sync.dma_start(out=gb_sb, in_=gb_d[:, :, :].transpose([2, 0, 1]))
        # prep stationaries: ob[:, g, :] is e_g x ones(128) -> broadcast row g
        ob_sb = cons.tile([4, 4, 128], bf16, tag="ob")
        nc.sync.dma_start(out=ob_sb, in_=ob_d[:, :, :])

        def load_weights(dram, nsl, cols, tag):
            wt = wts.tile([C, nsl, cols], bf16, tag=tag, name=f"wt_{tag}")
            nc.sync.dma_start(out=wt, in_=dram[:, :, :])
            return wt

        # ---------------- generic helpers ----------------
        def all_reduce(sb_in, k, groups, p=C):
            n = len(_ar_cnt)
            _ar_cnt.append(0)
            din = drp.tile([p, k], f32, tag=f"arin{n}")
            dout = drp.tile([p, k], f32, tag=f"arout{n}")
            nc.sync.dma_start(out=din, in_=sb_in)
            nc.gpsimd.collective_compute(
                "AllReduce", AL.add, replica_groups=groups,
                ins=[din[:, :].opt()], outs=[dout[:, :].opt()])
            sb_out = smc.tile([p, k], f32, tag=f"ar{n}")
            nc.sync.dma_start(out=sb_out, in_=dout)
            return sb_out

        _ar_cnt = []

        def sums_from_mv(mv, count, p=C):
            """mv [p,2] (mean, biased var) -> packed [p,2] (sum, sum_sq)."""
            pk = smc.tile([p, 2], f32, tag=f"pk{len(_pk_cnt)}")
            _pk_cnt.append(0)
            nc.vector.tensor_scalar_mul(out=pk[:, 0:1], in0=mv[:, 0:1],
                                        scalar1=float(count))
            # e2 = (var + mean^2) * count
            nc.vector.tensor_mul(out=pk[:, 1:2], in0=mv[:, 0:1], in1=mv[:, 0:1])
            nc.vector.tensor_add(out=pk[:, 1:2], in0=pk[:, 1:2], in1=mv[:, 1:2])
            nc.vector.tensor_scalar_mul(out=pk[:, 1:2], in0=pk[:, 1:2],
                                        scalar1=float(count))
            return pk

        def mu_rstd_from_sums(gl, total, p=C):
            """gl [p,2] global (sum, sumsq) -> (mu [p,1], rstd [p,1])."""
            n = len(_mr_cnt)
            _mr_cnt.append(0)
            mu = smc.tile([p, 1], f32, tag=f"mu{n}")
            rs = smc.tile([p, 1], f32, tag=f"rs{n}")
            tv = smc.tile([p, 1], f32, tag=f"tv{n}")
            nc.vector.tensor_scalar_mul(out=mu, in0=gl[:, 0:1],
                                        scalar1=1.0 / total)
            nc.vector.tensor_scalar(out=tv, in0=mu, scalar1=mu,
                                    scalar2=-1.0, op0=AL.mult, op1=AL.mult)
            nc.vector.tensor_scalar(out=tv, in0=gl[:, 1:2],
                                    scalar1=1.0 / total, scalar2=tv,
                                    op0=AL.mult, op1=AL.add)   # var
            nc.scalar.activation(out=tv, in_=tv, func=AF.Sqrt,
                                 bias=eps_sb[0:p, :], scale=1.0)
            nc.vector.reciprocal(out=rs, in_=tv)
            return mu, rs

        _pk_cnt = []
        _mr_cnt = []

        def bn_flat_stats(src_f32, p, flat_start, flat_len):
            """bn_stats over a contiguous flat span (pads must be zeroed;
            zeros only dilute mean/var, raw sums are unaffected)."""
            sf = flat(src_f32)
            nchunks = (flat_len + 511) // 512
            stats = sm.tile([p, nchunks, 6], f32, tag="st")
            for j in range(nchunks):
                a = flat_start + 512 * j
                b = min(flat_start + flat_len, a + 512)
                nc.vector.bn_stats(out=stats[:, j, :], in_=sf[0:p, a:b])
            mv = sm.tile([p, 2], f32, tag="mv")
            nc.vector.bn_aggr(out=mv, in_=stats)
            return mv

        def _build_body():
            # ================= stage 0: input build =================
            # reflect-pad layout built by direct strided DMAs; zeros only on
            # the border cells the shifted conv reads can touch. Input DMAs
            # are split across chunks so conv1 can start on the first rows.
            xp = stage.tile([C, 87, ST], bf16, tag="stage")
            nc.vector.memset(xp[:, 0:1, :], 0.0)
            nc.vector.memset(xp[:, 86:87, :], 0.0)
            nc.vector.memset(xp[:, 1:86, 0:1], 0.0)
            nc.vector.memset(xp[:, 1:86, 131:138], 0.0)
            w1_sb = load_weights(w1_d, 9, C, "w1")
            nc.sync.dma_start(out=xp[:, 1:2, 2:130], in_=xs_d[:, 1:2, :])
            nc.sync.dma_start(out=xp[:, 1:2, 1:2], in_=xs_d[:, 1:2, 1:2])
            nc.sync.dma_start(out=xp[:, 1:2, 130:131],
                              in_=xs_d[:, 1:2, 126:127])
            for r0 in range(0, 84, 21):
                r1 = r0 + 21
                nc.sync.dma_start(out=xp[:, 2 + r0:2 + r1, 2:130],
                                  in_=xs_d[:, r0:r1, :])
                nc.sync.dma_start(out=xp[:, 2 + r0:2 + r1, 1:2],
                                  in_=xs_d[:, r0:r1, 1:2])
                nc.sync.dma_start(out=xp[:, 2 + r0:2 + r1, 130:131],
                                  in_=xs_d[:, r0:r1, 126:127])

            # ====== preload every weight + the dct-mean vector right after
            # the input: mid-kernel DMA descriptor streams throttle the PE's
            # SBUF reads, so all bulk loads happen in the startup window.
            w2_sb = load_weights(w2_d, 9, C, "w2")
            s1_sb = load_weights(s_d[0], 14, 128, "s1")
            s2_sb = load_weights(s_d[1], 7, 128, "s2")
            s3_sb = load_weights(s_d[2], 28, 128, "s3")
            s4_sb = load_weights(s_d[3], 49, 128, "s4")
            s5_sb = load_weights(s_d[4], 28, 128, "s5")
            s6_sb = load_weights(s_d[5], 7, 128, "s6")
            # dct-mean vector also loads in the startup window: putting it in
            # the IN1-AR stall clogged the DMA path and delayed the collective
            # input by ~7us.
            wvfull = fpool.tile([C, HB, H2], bf16, tag="wvfull")
            nc.sync.dma_start(out=wvfull,
                              in_=wv_d[:, :].partition_broadcast(C))
            if debug_taps:
                nc.sync.dma_start(out=taps["xp"][:, :, :], in_=xp)
            ckpt(0)

            # ================= conv1 =================
            # weight-major groups: consecutive matmuls share the stationary
            # operand.
            out1 = stage.tile([C, 84, ST], bf16, tag="stage")
            nc.vector.memset(out1[:, :, 130:138], 0.0)
            xp_f = flat(xp)
            N1 = 414
            st1 = sm.tile([C, 22, 6], f32, tag="st")

            def conv1_group(ts):
                pts = [cps.tile([C, N1], f32, tag="cps", name=f"pt{t}")
                        for t in ts]
                for i in range(9):
                    di, dj = divmod(i, 3)
                    for k, t in enumerate(ts):
                        ob = 3 * t * ST
                        o = ob + di * ST + dj
                        nc.tensor.matmul(out=pts[k], lhsT=w1_sb[:, i, :],
                                         rhs=xp_f[:, o:o + N1],
                                         start=(i == 0), stop=(i == 8))
                for k, t in enumerate(ts):
                    ob = 3 * t * ST
                    ptv = pts[k].rearrange("p (r c) -> p r c", c=ST)
                    nc.scalar.copy(out=out1[:, 3 * t:3 * t + 3, 0:130],
                                   in_=ptv[:, :, 0:130])
                    # IN1 stats cover rows 0..64 only (65 rows)
                    if t < 21:
                        nc.vector.bn_stats(out=st1[:, t, :],
                                           in_=flat(out1)[:, ob:ob + N1])
                    elif t == 21:
                        nc.vector.bn_stats(out=st1[:, 21, :],
                                           in_=flat(out1)[:, 63 * ST:65 * ST])

            for g in ([0, 1, 2, 3, 4, 5], [6, 7, 8, 9, 10, 11],
                      [12, 13, 14, 15, 16, 17], [18, 19], [20, 21]):
                conv1_group(g)
            mv1 = sm.tile([C, 2], f32, tag="mv")
            nc.vector.bn_aggr(out=mv1, in_=st1)
            pk1 = sums_from_mv(mv1, 65 * ST)
            gl1 = all_reduce(pk1, 2, PAIRS)
            conv1_group([22, 23, 24, 25, 26, 27])
            mu1, rs1 = mu_rstd_from_sums(gl1, CNT1_TOT)
            if debug_taps:
                nc.sync.dma_start(out=taps["out1"][:, :, :], in_=out1)
            ckpt(1)
            if debug_taps:
                nc.sync.dma_start(out=taps["stats1"][:, :], in_=gl1)
            ckpt(2)

            # negated bias for ACT: relu(x*rs1 - mu1*rs1)
            nb1 = smc.tile([C, 1], f32, tag="nb1")
            nc.vector.tensor_scalar(out=nb1, in0=mu1, scalar1=rs1,
                                    scalar2=-1.0, op0=AL.mult, op1=AL.mult)

            # ================= r1p build (banded) =================
            r1p = stage.tile([C, 88, ST], bf16, tag="stage")
            nc.vector.memset(r1p[:, 0:2, :], 0.0)
            nc.vector.memset(r1p[:, 2:87, 0:1], 0.0)
            nc.vector.memset(r1p[:, 2:87, 133:138], 0.0)

            def rel(dst, src):
                nc.scalar.activation(out=dst, in_=src, func=AF.Relu,
                                     bias=nb1, scale=rs1)

            rel(r1p[:, 2:3, 1:2], out1[:, 1:2, 1:2])
            rel(r1p[:, 2:3, 2:132], out1[:, 1:2, 0:130])
            rel(r1p[:, 2:3, 132:133], out1[:, 1:2, 128:129])
            rel(r1p[:, 3:10, 1:2], out1[:, 0:7, 1:2])
            rel(r1p[:, 3:10, 132:133], out1[:, 0:7, 128:129])
            rel(r1p[:, 3:10, 2:132], out1[:, 0:7, 0:130])
            rel(r1p[:, 10:24, 2:132], out1[:, 7:21, 0:130])
            rel(r1p[:, 10:30, 1:2], out1[:, 7:27, 1:2])
            rel(r1p[:, 10:30, 132:133], out1[:, 7:27, 128:129])
            for a in range(21, 84, 21):
                rel(r1p[:, 3 + a:3 + a + 21, 2:132],
                    out1[:, a:a + 21, 0:130])
            rel(r1p[:, 30:87, 1:2], out1[:, 27:84, 1:2])
            rel(r1p[:, 30:87, 132:133], out1[:, 27:84, 128:129])
            if debug_taps:
                nc.sync.dma_start(out=taps["r1p"][:, :, :], in_=r1p)
            ckpt(3)

            # ================= conv2 (+ dct-mean dot per tile) ============
            out2 = stage.tile([C, 84, ST], bf16, tag="stage")
            nc.vector.memset(out2[:, :, 0:3], 0.0)
            nc.vector.memset(out2[:, :, 135:138], 0.0)
            r1_f = flat(r1p)
            st2 = sm.tile([C, 22, 6], f32, tag="st")
            acc = sm.tile([C, 22], f32, tag="dotacc")

            def conv2_group(ts):
                pts = [cps.tile([C, N1], f32, tag="cps", name=f"pt{t}")
                        for t in ts]
                for i in range(9):
                    di, dj = divmod(i, 3)
                    off = (di + 1) * ST + (dj - 3)
                    for k, t in enumerate(ts):
                        o = 3 * t * ST + off
                        nc.tensor.matmul(out=pts[k], lhsT=w2_sb[:, i, :],
                                         rhs=r1_f[:, o:o + N1],
                                         start=(i == 0), stop=(i == 8))
                for k, t in enumerate(ts):
                    ob = 3 * t * ST
                    ptv = pts[k].rearrange("p (r c) -> p r c", c=ST)
                    nc.scalar.copy(out=out2[:, 3 * t:3 * t + 3, 3:135],
                                   in_=ptv[:, :, 3:135])
                    if t < 22:
                        nc.vector.bn_stats(out=st2[:, t, :],
                                           in_=flat(out2)[:, ob:ob + N1])
                        scr = wvp.tile([C, 3, H2], f32, tag="scr")
                        nc.vector.tensor_mul(out=scr,
                                             in0=out2[:, 3 * t:3 * t + 3, 3:135],
                                             in1=wvfull[:, 3 * t:3 * t + 3, :])
                        nc.vector.tensor_reduce(out=acc[:, t:t + 1], in_=scr,
                                                axis=AX.XY, op=AL.add)

            for g in ([0, 1], [2, 3, 4, 5, 6, 7], [8, 9, 10, 11, 12, 13],
                      [14, 15, 16, 17, 18, 19], [20, 21]):
                conv2_group(g)
            mv2 = sm.tile([C, 2], f32, tag="mv")
            nc.vector.bn_aggr(out=mv2, in_=st2)
            dotw = smc.tile([C, 1], f32, tag="dotw")
            nc.vector.tensor_reduce(out=dotw, in_=acc, axis=AX.X, op=AL.add)
            pk2 = sums_from_mv(mv2, 66 * ST)
            pk2b = smc.tile([C, 3], f32, tag="pk2b")
            nc.vector.tensor_copy(out=pk2b[:, 0:2], in_=pk2)
            nc.vector.tensor_copy(out=pk2b[:, 2:3], in_=dotw)
            gl2 = all_reduce(pk2b, 3, PAIRS)
            conv2_group([22, 23, 24, 25, 26, 27])
            mu2, rs2 = mu_rstd_from_sums(gl2, CNT2_TOT)
            if debug_taps:
                nc.sync.dma_start(out=taps["out2"][:, :, :], in_=out2)
            ckpt(4)
            if debug_taps:
                nc.sync.dma_start(out=taps["stats2"][:, :], in_=gl2)

            # ================= SFOM gate =================
            # m = rs2 * (dotw_glob - mu2 * S_w)
            m_sb = smc.tile([C, 1], f32, tag="m")
            nc.vector.tensor_scalar(out=m_sb, in0=mu2, scalar1=-S_W,
                                    scalar2=gl2[:, 2:3], op0=AL.mult,
                                    op1=AL.add)
            nc.vector.tensor_mul(out=m_sb, in0=m_sb, in1=rs2)
            # gate = sigmoid(relu(m @ sa1) @ sa2)
            p_r = cps.tile([8, 1], f32, tag="cps", name="p_r")
            nc.tensor.matmul(out=p_r, lhsT=sa1_sb, rhs=m_sb, start=True, stop=True)
            relu_sb = smc.tile([8, 1], f32, tag="relu8")
            nc.scalar.activation(out=relu_sb, in_=p_r, func=AF.Relu,
                                 bias=0.0, scale=1.0)
            p_g = cps.tile([C, 1], f32, tag="cps", name="p_g")
            nc.tensor.matmul(out=p_g, lhsT=sa2_sb, rhs=relu_sb,
                             start=True, stop=True)
            gate = smc.tile([C, 1], f32, tag="gate")
            nc.scalar.activation(out=gate, in_=p_g, func=AF.Sigmoid,
                                 bias=0.0, scale=1.0)
            if debug_taps:
                nc.sync.dma_start(out=taps["gate"][:, :], in_=gate)
            ckpt(5)
            # s_sig = rs2 * (1+gate)/2
            ssig = smc.tile([C, 1], f32, tag="ssig")
            nc.vector.tensor_scalar(out=ssig, in0=gate, scalar1=0.5, scalar2=0.5,
                                    op0=AL.mult, op1=AL.add)
            nc.vector.tensor_mul(out=ssig, in0=ssig, in1=rs2)
            nbs = smc.tile([C, 1], f32, tag="nbs")     # -mu2*ssig
            nc.vector.tensor_scalar(out=nbs, in0=mu2, scalar1=ssig,
                                    scalar2=-1.0, op0=AL.mult, op1=AL.mult)

            # ================= SFOM apply (banded) =================
            # o2 = (out2-mu2)*rs2 ; f = sigmoid(o2*g2')*o2  (slots +4 rows)
            o2 = stage.tile([C, 84, ST], bf16, tag="stage")
            ftile = fpool.tile([C, R_F + 5, ST], bf16, tag="f")
            fr = ftile
            nc.vector.memset(ftile[:, 0:4, :], 0.0)
            nc.vector.memset(ftile[:, 88:89, :], 0.0)
            nc.vector.memset(ftile[:, 4:88, 0:3], 0.0)
            nc.vector.memset(ftile[:, 4:88, 135:138], 0.0)
            fbands = [0, 10, 21, 42, 63, 84]
            for a, e in zip(fbands, fbands[1:]):
                nc.vector.tensor_scalar(out=o2[:, a:e, 3:135],
                                        in0=out2[:, a:e, 3:135],
                                        scalar1=mu2, scalar2=rs2,
                                        op0=AL.subtract, op1=AL.mult)
                nc.scalar.activation(out=fr[:, 4 + a:4 + e, 3:135],
                                     in_=out2[:, a:e, 3:135],
                                     func=AF.Sigmoid, bias=nbs, scale=ssig)
                nc.vector.tensor_mul(out=fr[:, 4 + a:4 + e, 3:135],
                                     in0=ftile[:, 4 + a:4 + e, 3:135],
                                     in1=o2[:, a:e, 3:135])
            if debug_taps:
                nc.sync.dma_start(out=taps["f"][:, :, :], in_=ftile)
            ckpt(6)

            # ================= SPEM layers =================
            def spem_layer(lidx, src_r, wtile, co, R, NP, mms, combine, ncopies,
                           copy_cp, post_band=None):
                """One SPEM conv layer: weight-major matmul groups, strided
                psum drains, per-tile BN stats, early AR, banded relu +
                K-pack copies."""
                S = R + 5
                P = 128 if (ncopies or co > 64) else co
                h = stage.tile([P, S, ST], bf16, tag="stage")
                nc.vector.memset(h[:, 0:4, :], 0.0)
                nc.vector.memset(h[:, 4 + R:S, :], 0.0)
                nc.vector.memset(h[:, 4:4 + R, 0:3], 0.0)
                nc.vector.memset(h[:, 4:4 + R, 135:138], 0.0)
                src_f = flat(src_r)
                ntiles = R // 3
                st = sm.tile([co, 22, 6], f32, tag="st")

                def conv_group(ts):
                    pts = [cps.tile([128, NP], f32, tag="cps", name=f"pt{t}")
                            for t in ts]
                    for i, (sl, beta) in enumerate(mms):
                        for k, t in enumerate(ts):
                            o = (4 + 3 * t) * ST + beta
                            nc.tensor.matmul(out=pts[k], lhsT=wtile[:, sl, :],
                                             rhs=src_f[:, o:o + NP],
                                             start=(i == 0),
                                             stop=(i == len(mms) - 1))
                    for k, t in enumerate(ts):
                        combine(pts[k], h, t)
                        if t < 22:
                            ob = (4 + 3 * t) * ST
                            nc.vector.bn_stats(
                                out=st[:, t, :],
                                in_=flat(h)[0:co, ob:ob + N1])

                for g in ([0, 1, 2, 3, 4, 5], [6, 7, 8, 9, 10, 11],
                          [12, 13, 14, 15, 16, 17], [18, 19], [20, 21]):
                    conv_group(g)
                mvb = sm.tile([co, 2], f32, tag="mv")
                nc.vector.bn_aggr(out=mvb, in_=st)
                pkb = sums_from_mv(mvb, 66 * ST, p=co)
                glb = all_reduce(pkb, 2, ALL8, p=co)
                if ntiles > 22:
                    conv_group(list(range(22, ntiles)))
                mub, rsb = mu_rstd_from_sums(glb, CNTB_TOT, p=co)
                # scale = gamma*rstd ; bias = beta - mu*scale
                sc = smc.tile([co, 1], f32, tag=f"sc{lidx}")
                bi = smc.tile([co, 1], f32, tag=f"bi{lidx}")
                nc.vector.tensor_mul(out=sc, in0=gb_sb[0:co, lidx, 0:1], in1=rsb)
                nc.vector.tensor_mul(out=bi, in0=mub, in1=sc)
                nc.vector.tensor_sub(out=bi, in0=gb_sb[0:co, lidx, 1:2], in1=bi)
                # banded relu + K-pack shifted copies (+ optional hook);
                # short first band so the next layer's matmuls start sooner
                bands = [4, 10] + list(range(25, 4 + R, 15)) + [4 + R]
                for a, e in zip(bands, bands[1:]):
                    nc.scalar.activation(out=h[0:co, a:e, 3:135],
                                         in_=h[0:co, a:e, 3:135],
                                         func=AF.Relu, bias=bi, scale=sc)
                    for g2 in range(1, ncopies + 1):
                        nc.vector.tensor_copy(
                            out=h[g2 * copy_cp:(g2 + 1) * copy_cp, a:e,
                                  0:ST - g2],
                            in_=h[0:copy_cp, a:e, g2:ST])
                    if post_band is not None:
                        post_band(h, e)
                if debug_taps:
                    tp = taps[f"h{lidx+1}"]
                    nc.sync.dma_start(out=tp[:, :, :], in_=h[0:tp.shape[0], :, :])
                return h

            def drain_act(pt, h, t):
                ptv = pt.rearrange("p (r c) -> p r c", c=ST)
                nc.scalar.copy(out=h[:, 4 + 3 * t:7 + 3 * t, 3:135],
                               in_=ptv[:, :, 3:135])

            def mk_combine(groups, cp):
                """groups: list of (psum partition group idx, col shift).
                DVE reads at most one PSUM operand: copy then accumulate."""
                def comb(pt, h, t):
                    r0 = 4 + 3 * t
                    g0, s0 = groups[0]
                    v0 = pt[g0 * cp:(g0 + 1) * cp, s0:s0 + N1].rearrange(
                        "p (r c) -> p r c", c=ST)
                    nc.scalar.copy(out=h[0:cp, r0:r0 + 3, 3:135],
                                   in_=v0[:, :, 3:135])
                    for g, s in groups[1:]:
                        v = pt[g * cp:(g + 1) * cp, s:s + N1].rearrange(
                            "p (r c) -> p r c", c=ST)
                        nc.vector.tensor_add(out=h[0:cp, r0:r0 + 3, 3:135],
                                             in0=h[0:cp, r0:r0 + 3, 3:135],
                                             in1=v[:, :, 3:135])
                return comb

            # L1: 128->32, Mpack4: psum[g*32+co] <-> out[n-g]
            mms1 = [(di * 2 + s, (di - 3) * ST + 4 * s - 3)
                    for di in range(7) for s in range(2)]
            h1 = spem_layer(0, fr, s1_sb, 32, R_H[0], 418, mms1,
                            mk_combine([(0, 0), (1, 1), (2, 2), (3, 3)], 32),
                            3, 32)
            # L2: 32->64, Kpack4 + Mpack2(supergroups +4): psum[G*64+co]<->out[n-4G]
            ckpt(7)
            mms2 = [(di, (di - 3) * ST - 3) for di in range(7)]
            h2 = spem_layer(1, h1, s2_sb, 64, R_H[1], 418, mms2,
                            mk_combine([(0, 0), (1, 4)], 64), 1, 64)
            # L3: 64->128, Kpack2: 4 dj-groups
            ckpt(8)
            mms3 = [(di * 4 + g, (di - 3) * ST + 2 * g - 3)
                    for di in range(7) for g in range(4)]
            h3 = spem_layer(2, h2, s3_sb, 128, R_H[2], 414, mms3, drain_act,
                            0, 0)
            # L4: 128->128 plain
            ckpt(9)
            mms4 = [(di * 7 + dj, (di - 3) * ST + dj - 3)
                    for di in range(7) for dj in range(7)]
            h4 = spem_layer(3, h3, s4_sb, 128, R_H[3], 414, mms4, drain_act,
                            0, 0)
            # L5: 128->64, Mpack2: psum[g*64+co] <-> out[n-g]
            ckpt(10)
            mms5 = [(di * 4 + st_, (di - 3) * ST + 2 * st_ - 3)
                    for di in range(7) for st_ in range(4)]
            h5 = spem_layer(4, h4, s5_sb, 64, R_H[4], 416, mms5,
                            mk_combine([(0, 0), (1, 1)], 64), 1, 64)
            # ===== L6: 64->32, Kpack2 + Mpack4, output STACKED x4 =====
            # h6s[32*g + c, T, :] holds tile t = 4*T + g (3 rows x 138 flat).
            # Stacking 4 tiles across the partition axis makes relu/sigmoid/
            # stats 4x cheaper and lets one matmul compute 4 tiles' 1x1 conv.
            ckpt(11)
            mms6 = [(di, (di - 3) * ST - 3) for di in range(7)]
            SL6 = 6
            h6s = stage.tile([C, SL6, 414], bf16, tag="stage")
            h6v = h6s.rearrange("p s (r c) -> p s r c", c=ST)
            nc.vector.memset(h6v[:, :, :, 0:3], 0.0)
            nc.vector.memset(h6v[:, :, :, 135:138], 0.0)
            nc.vector.memset(h6s[64:128, 5, :], 0.0)
            src6_f = flat(h5)
            st6 = sm.tile([C, SL6, 6], f32, tag="st")

            def l6_group(ts, T0, T1):
                pts = [cps.tile([128, 420], f32, tag="cps", name=f"pt{t}")
                        for t in ts]
                for i, (sl, beta) in enumerate(mms6):
                    for k, t in enumerate(ts):
                        o = (4 + 3 * t) * ST + beta
                        nc.tensor.matmul(out=pts[k], lhsT=s6_sb[:, sl, :],
                                         rhs=src6_f[:, o:o + 420],
                                         start=(i == 0), stop=(i == 6))
                for k, t in enumerate(ts):
                    g, T = t % 4, t // 4
                    dst = h6v[32 * g:32 * g + 32, T, :, 3:135]
                    v0 = pts[k][0:32, 0:414].rearrange(
                        "p (r c) -> p r c", c=ST)
                    nc.scalar.copy(out=dst, in_=v0[:, :, 3:135])
                    for gg, s in [(1, 2), (2, 4), (3, 6)]:
                        v = pts[k][32 * gg:32 * gg + 32, s:s + 414].rearrange(
                            "p (r c) -> p r c", c=ST)
                        nc.vector.tensor_add(out=dst, in0=dst,
                                             in1=v[:, :, 3:135])
                for T in range(T0, T1):
                    nc.vector.bn_stats(out=st6[:, T, :], in_=h6s[:, T, :])

            l6_group([0, 1, 2, 3, 4, 5], 0, 1)
            l6_group([6, 7, 8, 9, 10, 11], 1, 3)
            l6_group([12, 13, 14, 15, 16, 17], 3, 4)
            l6_group([18, 19], 4, 5)
            l6_group([20, 21], 5, 6)
            mv6 = sm.tile([C, 2], f32, tag="mv")
            nc.vector.bn_aggr(out=mv6, in_=st6)
            pk6 = sums_from_mv(mv6, SL6 * 414, p=C)
            pkf = smc.tile([32, 3, 2], f32, tag="pkf")
            for g in range(1, 4):
                nc.vector.tensor_copy(out=pkf[:, g - 1, :],
                                      in_=pk6[32 * g:32 * g + 32, :])
            pk32 = smc.tile([32, 2], f32, tag="pk32")
            nc.vector.tensor_add(out=pk32, in0=pk6[0:32, :], in1=pkf[:, 0, :])
            nc.vector.tensor_add(out=pk32, in0=pk32, in1=pkf[:, 1, :])
            nc.vector.tensor_add(out=pk32, in0=pk32, in1=pkf[:, 2, :])
            gl6 = all_reduce(pk32, 2, ALL8, p=32)
            mu6, rs6 = mu_rstd_from_sums(gl6, CNTB_TOT, p=32)
            sc6 = smc.tile([32, 1], f32, tag="sc6")
            bi6 = smc.tile([32, 1], f32, tag="bi6")
            nc.vector.tensor_mul(out=sc6, in0=gb_sb[0:32, 5, 0:1], in1=rs6)
            nc.vector.tensor_mul(out=bi6, in0=mu6, in1=sc6)
            nc.vector.tensor_sub(out=bi6, in0=gb_sb[0:32, 5, 1:2], in1=bi6)
            sc6s = smc.tile([C, 1], f32, tag="sc6s")
            bi6s = smc.tile([C, 1], f32, tag="bi6s")
            for g in range(4):
                nc.vector.tensor_copy(out=sc6s[32 * g:32 * g + 32], in_=sc6)
                nc.vector.tensor_copy(out=bi6s[32 * g:32 * g + 32], in_=bi6)

            # relu + 1x1 conv (4 tiles per matmul) + sigmoid; separate loops
            # keep each engine streaming instead of ACT<->PE ping-pong
            sa_s = smc.tile([4, SL6, 414], bf16, tag="sa_s")
            p7s = []
            for T in range(SL6):
                nc.scalar.activation(out=h6v[:, T, :, 3:135],
                                     in_=h6v[:, T, :, 3:135],
                                     func=AF.Relu, bias=bi6s, scale=sc6s)
            for T in range(SL6):
                p7 = cps.tile([4, 414], f32, tag="cps", name=f"p7_{T}")
                nc.tensor.matmul(out=p7, lhsT=cw_sb, rhs=h6s[:, T, :],
                                 start=True, stop=True)
                p7s.append(p7)
            for T in range(SL6):
                nc.scalar.activation(out=sa_s[:, T, :], in_=p7s[T],
                                     func=AF.Sigmoid, bias=cb_sb, scale=1.0)

            if debug_taps:
                nc.sync.dma_start(out=taps["h6"][:, :, :], in_=h6s)

            # broadcast + multiply + store; stationary grouped by g
            f_f = flat(ftile)
            for g in range(4):
                for T in range(6 if g < 2 else 5):
                    t = 4 * T + g
                    obs = (4 + 3 * t) * ST
                    prep = cps.tile([128, 414], f32, tag="cps",
                                    name=f"prep_{t}")
                    nc.tensor.matmul(out=prep, lhsT=ob_sb[:, g, :],
                                     rhs=sa_s[:, T, :], start=True, stop=True)
                    ot = otp.tile([C, 414], f32, tag="ot")
                    nc.vector.tensor_mul(out=ot, in0=prep,
                                         in1=f_f[:, obs:obs + 414])
                    nc.sync.dma_start(out=out_d[:, t, :], in_=ot)


        try:
            _build_body()
        except _StopBuild:
            pass
        for p in [drp, cps, sap, otp, wvp, smc, sm, cons, wts, fpool,
                  stage]:
            p.release()

    nc.compile()
    _PROGRAM_CACHE[key] = (nc, taps)
    return nc, taps


# ----------------------------------------------------------------------------
# host-side packing
# ----------------------------------------------------------------------------

def _pack_core_inputs(inputs, core):
    b, half = core // 2, core % 2
    flip = (half == 1)

    def fd(w):          # flip di (axis 2) of [co, ci, kh, kw]
        return w[:, :, ::-1, :] if flip else w

    x = inputs['x'][b]
    if flip:
        x = x[:, ::-1, :]
    xs = np.ascontiguousarray(x[:, 0:XROWS, :]).astype(BF16)

    w1 = fd(inputs['conv1_w'])
    w2 = fd(inputs['conv2_w'])
    w1p = np.ascontiguousarray(
        np.transpose(w1, (2, 3, 1, 0)).reshape(9, C, C).transpose(1, 0, 2),
        np.float32)
    w2p = np.ascontiguousarray(
        np.transpose(w2, (2, 3, 1, 0)).reshape(9, C, C).transpose(1, 0, 2),
        np.float32)

    dws = [fd(inputs[f'dw{i}']) for i in range(1, 7)]

    def lhsT(w, di, dj):
        return w[:, :, di, dj].T        # [ci, co]

    # s1 [C, 14, 128]: idx di*2+s ; cols g*32+co = dj=4s+g
    s1 = np.zeros((C, 14, 128), np.float32)
    for di in range(7):
        for s in range(2):
            for g in range(4):
                dj = 4 * s + g
                if dj < 7:
                    s1[:, di * 2 + s, 32 * g:32 * g + 32] = lhsT(dws[0], di, dj)
    # s2 [C, 7, 128]: idx di ; rows s*32+ci ; cols G*64+co = dj=4G+s
    s2 = np.zeros((C, 7, 128), np.float32)
    for di in range(7):
        for s in range(4):
            for G in range(2):
                dj = 4 * G + s
                if dj < 7:
                    s2[32 * s:32 * s + 32, di, 64 * G:64 * G + 64] = \
                        lhsT(dws[1], di, dj)
    # s3 [C, 28, 128]: idx di*4+grp ; rows s*64+ci ; dj=2grp+s
    s3 = np.zeros((C, 28, 128), np.float32)
    for di in range(7):
        for grp in range(4):
            for s in range(2):
                dj = 2 * grp + s
                if dj < 7:
                    s3[64 * s:64 * s + 64, di * 4 + grp, :] = \
                        lhsT(dws[2], di, dj)
    # s4 [C, 49, 128]
    s4 = np.zeros((C, 49, 128), np.float32)
    for di in range(7):
        for dj in range(7):
            s4[:, di * 7 + dj, :] = lhsT(dws[3], di, dj)
    # s5 [C, 28, 128]: idx di*4+st ; cols g*64+co = dj=2st+g
    s5 = np.zeros((C, 28, 128), np.float32)
    for di in range(7):
        for st in range(4):
            for g in range(2):
                dj = 2 * st + g
                if dj < 7:
                    s5[:, di * 4 + st, 64 * g:64 * g + 64] = \
                        lhsT(dws[4], di, dj)
    # s6 [C, 7, 128]: idx di ; rows s*64+ci ; cols G*32+co = dj=2G+s
    s6 = np.zeros((C, 7, 128), np.float32)
    for di in range(7):
        for G in range(4):
            for s in range(2):
                dj = 2 * G + s
                if dj < 7:
                    s6[64 * s:64 * s + 64, di,
                       32 * G:32 * G + 32] = lhsT(dws[5], di, dj)

    # cw block-diagonal for the stacked 1x1 conv: cwb[32g+c, g] = cw[c]
    cwv = np.asarray(inputs['spem_cw'][0, :, 0, 0], np.float32)
    cwb = np.zeros((C, 4), np.float32)
    obk = np.zeros((4, 4, 128), np.float32)
    for g in range(4):
        cwb[32 * g:32 * g + 32, g] = cwv
        obk[g, g, :] = 1.0

    wvec = dct_mean_weights().reshape(H2, H2)
    if flip:
        wv = np.ascontiguousarray(wvec[::-1, :][0:HB]).astype(BF16)
    else:
        wv = np.ascontiguousarray(wvec[0:HB]).astype(BF16)

    gb = np.zeros((6, 2, C), np.float32)
    for k in range(6):
        g = inputs[f'bg{k+1}']
        bb = inputs[f'bb{k+1}']
        gb[k, 0, :len(g)] = g
        gb[k, 1, :len(bb)] = bb

    return {
        'xs': xs, 'w1': w1p.astype(BF16), 'w2c': w2p.astype(BF16),
        's1': s1.astype(BF16), 's2': s2.astype(BF16), 's3': s3.astype(BF16),
        's4': s4.astype(BF16), 's5': s5.astype(BF16), 's6': s6.astype(BF16),
        'cw': cwb.astype(BF16),
        'cb': np.full((4, 1), float(np.asarray(inputs['spem_cb']).reshape(())),
                      np.float32),
        'obk': obk.astype(BF16),
        'sa1t': np.ascontiguousarray(inputs['sa_w1'].T, np.float32),
        'sa2t': np.ascontiguousarray(inputs['sa_w2'].T, np.float32),
        'gb': gb, 'wv': wv,
    }


def run_cores(inputs, trace=False, debug_taps=False, stage_limit=99):
    _install_ntff_hook()
    from concourse.bass_utils import run_bass_kernel_spmd
    nc, taps = build_program(debug_taps=debug_taps, stage_limit=stage_limit)
    in_maps = [_pack_core_inputs(inputs, c) for c in range(NCORES)]
    res = run_bass_kernel_spmd(nc, in_maps, list(range(NCORES)), trace=trace)
    return res


def unpack_out(o):
    """[C, 22, 414] flat tile rows -> [C, HB, H2]."""
    return np.asarray(o, np.float32).reshape(C, HB, ST)[:, :, 3:135]


def kernel(**inputs):
    res = run_cores(inputs)
    full = np.empty((B, C, H2, H2), np.float32)
    for b in range(B):
        full[b, :, 0:HB, :] = unpack_out(res.results[2 * b]["out"])
        full[b, :, HB:H2, :] = unpack_out(res.results[2 * b + 1]["out"])[:, ::-1, :]
    return full

